# revision 36
# baseline (speedup 1.0000x reference)
"""Trainium2 Bass kernel for nn_AttentionLayer (sparse_attention).

reference:
    S[i,o]   = sum_k key[b,i,0,k] * query[b,0,o,k] / sqrt(64)
    W        = softmax(S, axis=i)                      # weights output [B, NI, NO]
    vals     = einsum("bio,biv->bov", W, value)
    vals     = vals / (||vals||_2(axis=v) + 1e-12)
    values   = swapaxes(vals, 1, 2)                    # [B, V, NO]
    returns (values, W)

Sharding: data-parallel over batch, one batch element per NeuronCore (B=8).

Per-core pipeline (bf16 matmuls, f32 accumulation):
  - inputs cast to bf16 on-chip; K and V use a paired-row load (two DRAM rows
    per partition -> 512B descriptors at line rate), which permutes the i
    order; the weights-output DMA access pattern compensates.
  - KT2: K^T with k on partitions, even/odd-row tiles stacked in the two
    64-partition halves so the QK^T matmuls run two i-tiles concurrently via
    PE row tiling; QT2 = Q^T duplicated into both halves.
  - per o-chunk (4 x 512):
      per i-tile pair (8): S = 2 concurrent matmuls (K=64) -> [128, 2x512]
        PSUM (separate banks); U = exp(0.125*S) -> bf16 (one ACT op/pair);
        vals_ps += matmul(lhsT=V'[t], rhs=U[t])  ([65, 512]; the V' ones
        column makes row 64 the softmax denominator)
      recip = exp(-ln(colsum)) on ACT (stays in the exp table set);
      rep = ones-matmul broadcast of recip; W = U * rep (DVE bf16 2x) ->
      SWDGE DMA with bf16->f32 cast
      values normalization (rsqrt via exp(-0.5 ln)) runs one chunk behind,
      off the critical path; softmax denominators cancel inside the L2 norm.
  - dummy filler matmuls keep the PE activity monitor from re-throttling the
    clock to 1.2 GHz during ACT/DMA-bound stretches.
"""

import numpy as np

B, NI, NO, K, V = 8, 2048, 2048, 64, 64
P = 128          # partitions / i-tile height
T = NI // P      # 16 i-tiles
NP = T // 2      # 8 i-tile pairs
CH = 512         # o-chunk width (one PSUM bank of f32)
NCH = NO // CH   # 4 chunks
HALF = T // 2    # W scale/DMA granularity (half a chunk)
WARMUP_MM = 10

_CACHE = {}


def _build():
    import concourse.bass as bass
    import concourse.tile as tile
    from concourse import bacc, mybir
    from concourse.bass import _add_dep_helper
    from concourse.masks import make_identity

    f32 = mybir.dt.float32
    bf16 = mybir.dt.bfloat16
    AF = mybir.ActivationFunctionType

    nc = bacc.Bacc("TRN2", target_bir_lowering=False, debug=False, num_devices=8)
    key_d = nc.declare_dram_parameter("key", [NI, K], f32, isOutput=False)
    query_d = nc.declare_dram_parameter("query", [NO, K], f32, isOutput=False)
    value_d = nc.declare_dram_parameter("value", [NI, V], f32, isOutput=False)
    values_o = nc.declare_dram_parameter("values_out", [V, NO], f32, isOutput=True)
    weights_o = nc.declare_dram_parameter("weights_out", [NI, NO], f32, isOutput=True)

    with tile.TileContext(nc) as tc:
        with (
            tc.tile_pool(name="const", bufs=1) as constp,
            tc.tile_pool(name="stage", bufs=2) as stagep,
            tc.tile_pool(name="u", bufs=3) as up,
            tc.tile_pool(name="w", bufs=3) as wp,
            tc.tile_pool(name="small", bufs=3) as smallp,
            tc.tile_pool(name="ps_s", bufs=2, space="PSUM") as ps_s,
            tc.tile_pool(name="ps_v", bufs=2, space="PSUM") as ps_v,
            tc.tile_pool(name="ps_m", bufs=2, space="PSUM") as ps_m,
        ):
            # Pin the ACT spline table set to one containing Exp+Ln+Copy so the
            # compiler never has to switch sets mid-kernel (~2.7us per switch).
            try:
                from concourse.hw_specs import get_activation_tables

                tabs = get_activation_tables(nc.m.arch)
                want = {AF.Exp, AF.Ln, AF.Copy}
                set_id = next(
                    i for i, fns in enumerate(tabs.values()) if want <= fns
                )
                nc.scalar.add_instruction(
                    mybir.InstLoadActFuncSet(
                        name=nc.get_next_instruction_name(),
                        act_func_set_id=set_id,
                        ins=[],
                        outs=[],
                    )
                )
            except Exception:
                pass  # fall back to compiler-inserted table loads

            ident = constp.tile([P, P], f32)
            make_identity(nc, ident[:])
            ones_row = constp.tile([1, P], bf16)
            nc.vector.memset(ones_row[:], 1.0)
            ones_col = constp.tile([K, 1], bf16)
            nc.vector.memset(ones_col[:], 1.0)

            # wu_ps: dummy-matmul target; filler matmuls keep the PE HAM warm
            wu_src = constp.tile([K, CH], bf16)
            nc.vector.memset(wu_src[:], 0.0)
            wu_ps = ps_m.tile([P, CH], f32, tag="misc")

            def filler_mm(n=1):
                for _ in range(n):
                    nc.tensor.matmul(
                        wu_ps[:], wu_src[:, 0:P], wu_src[:], start=True, stop=True
                    )

            # K^T/Q^T bf16, duplicated across both 64-partition halves:
            #   KT2[0:64, pt*128:+128]  = K^T of even rows of pair-group pt
            #   KT2[64:128, pt*128:+128]= K^T of odd rows
            #   QT2[0:64, :] = Q^T ; QT2[64:128, :] = copy of Q^T
            kt2 = constp.tile([P, NP * P], bf16)
            qt2 = constp.tile([P, NO], bf16)

            qstg_f = stagep.tile([P, T * K], f32, tag="qstg_f")
            kstg_f = stagep.tile([P, T * K], f32, tag="kstg_f")
            kstg_f3 = kstg_f[:].rearrange("p (g f) -> p g f", g=NP)
            qstg_f3 = qstg_f[:].rearrange("p (t k) -> p t k", t=T)
            kv = key_d.ap().rearrange("(g p two) k -> p g (two k)", g=NP, p=P)
            qv = query_d.ap().rearrange("(t p) k -> p t k", p=P)
            HT = T // 2
            for qq in range(4):
                nc.sync.dma_start(
                    kstg_f3[:, 2 * qq : 2 * qq + 2, :],
                    kv[:, 2 * qq : 2 * qq + 2, :],
                )
            for hh in range(2):
                nc.scalar.dma_start(
                    qstg_f3[:, hh * HT : (hh + 1) * HT, :],
                    qv[:, hh * HT : (hh + 1) * HT, :],
                )

            # V' = [value | ones] bf16, per i-tile: [128, 65]
            vp = constp.tile([P, T * (V + 1)], bf16)
            vp3 = vp[:].rearrange("p (t c) -> p t c", t=T)
            nc.vector.memset(vp3[:, :, V : V + 1], 1.0)
            vstg = stagep.tile([P, T * V], f32, tag="vstg")
            nc.scalar.dma_start(
                vstg[:].rearrange("p (g f) -> p g f", g=NP),
                value_d.ap().rearrange("(g p two) v -> p g (two v)", g=NP, p=P),
            )

            def q_group(g):  # Q tiles 4g..4g+3 -> QT2 cols [512g, 512g+512)
                tp = ps_m.tile([K, 4 * P], f32, tag="misc")
                for j in range(4):
                    nc.tensor.transpose(
                        tp[:, j * P : (j + 1) * P], qstg_f3[:, 4 * g + j, :], ident[:]
                    )
                csl = slice(g * 4 * P, (g + 1) * 4 * P)
                nc.vector.tensor_copy(qt2[0:K, csl], tp[:])
                nc.vector.tensor_copy(qt2[K:P, csl], tp[:])

            def k_pair(pt):  # paired group pt -> KT2 col block pt (both halves)
                tp = ps_m.tile([P, P], f32, tag="misc")
                nc.tensor.transpose(tp[:], kstg_f3[:, pt, :], ident[:])
                nc.vector.tensor_copy(kt2[:, pt * P : (pt + 1) * P], tp[:])

            k_pair(0)
            k_pair(1)
            q_group(0)
            for qq in range(1, 4):
                k_pair(2 * qq)
                k_pair(2 * qq + 1)
            q_group(1)
            nc.vector.tensor_copy(
                vp3[:, :, 0:V], vstg[:].rearrange("p (t v) -> p t v", t=T)
            )
            q_group(2)
            q_group(3)

            vals_all = constp.tile([V, NO], f32)
            wo4 = weights_o.ap().rearrange("(g p two) o -> p g two o", g=NP, p=P)

            def values_block(span):
                # values = vals * rsqrt(sum_v vals^2); softmax denoms cancel
                o0b, cwb = span
                csl = slice(o0b, o0b + cwb)
                sq = smallp.tile([V, CH], bf16, tag="sq")
                nc.vector.tensor_mul(
                    sq[:, 0:cwb], vals_all[:, csl], vals_all[:, csl]
                )
                ss_ps = ps_m.tile([1, CH], f32, tag="misc")
                nc.tensor.matmul(
                    ss_ps[:, 0:cwb], ones_col[:, :], sq[:, 0:cwb],
                    start=True, stop=True,
                )
                lnt = smallp.tile([1, CH], f32, tag="lnt")
                nc.scalar.activation(lnt[:, 0:cwb], ss_ps[:, 0:cwb], AF.Ln)
                invn = smallp.tile([1, CH], bf16, tag="invn")
                nc.scalar.activation(
                    invn[:, 0:cwb], lnt[:, 0:cwb], AF.Exp, scale=-0.5
                )
                repn_ps = ps_m.tile([V, CH], f32, tag="misc")
                nc.tensor.matmul(
                    repn_ps[:, 0:cwb], ones_row[:, 0:V], invn[:, 0:cwb],
                    start=True, stop=True,
                )
                vouts = smallp.tile([V, CH], f32, tag="vouts")
                nc.vector.tensor_mul(
                    vouts[:, 0:cwb], vals_all[:, csl], repn_ps[:, 0:cwb]
                )
                nc.sync.dma_start(values_o.ap()[:, csl], vouts[:, 0:cwb])

            # chunk plan: narrow leading chunks so the weights DMA starts early
            CHUNKS = [(0, 256), (256, 256), (512, 512), (1024, 512), (1536, 512)]
            prev_recip = None
            for c, (o0, cw) in enumerate(CHUNKS):
                u = up.tile([P, T * cw], bf16, tag="u")
                u3 = u[:].rearrange("p (t n) -> p t n", t=T)
                vals_ps = ps_v.tile([V + 1, cw], f32, tag="vals")
                qsl = slice(o0, o0 + cw)
                for pt in range(NP):
                    # concurrent row-tiled matmuls must land in different PSUM
                    # banks: pin halves at CH-sized bank offsets even when
                    # cw < CH
                    s_ps = ps_s.tile([P, 2 * CH], f32, tag="s")
                    for j in range(2):
                        nc.tensor.matmul(
                            s_ps[:, j * CH : j * CH + cw],
                            kt2[j * K : (j + 1) * K, pt * P : (pt + 1) * P],
                            qt2[j * K : (j + 1) * K, qsl],
                            start=True,
                            stop=True,
                        )
                    exp_inst = nc.scalar.activation(
                        u3[:, 2 * pt : 2 * pt + 2, :],
                        s_ps[:].rearrange("p (j n) -> p j n", j=2)[:, :, 0:cw],
                        AF.Exp,
                        scale=0.125,
                    )
                    if pt == 0 and prev_recip is not None:
                        # keep the previous chunk's weights-DMA-gating recip
                        # ahead of this chunk's exps in the ACT stream
                        _add_dep_helper(
                            exp_inst.ins, prev_recip.ins,
                            sync=False, reason="recip before next-chunk exps",
                        )
                    for j in range(2):
                        t = 2 * pt + j
                        nc.tensor.matmul(
                            vals_ps[:],
                            vp3[:, t, :],
                            u3[:, t, :],
                            start=(t == 0),
                            stop=(t == T - 1),
                        )
                    if c >= 2 and pt % 2 == 1:
                        filler_mm()

                # weights = U * (1/colsum); recip via exp(-ln) stays in the exp
                # table set (ACT Reciprocal banned; DVE recip on 1 partition is
                # 3.4us). High priority: this chain gates the weights DMA, so
                # it must beat the next chunk's exps to the ACT queue.
                lncs = smallp.tile([1, CH], f32, tag="lncs")
                nc.scalar.activation(lncs[:, 0:cw], vals_ps[V : V + 1, :], AF.Ln)
                recip = smallp.tile([1, CH], bf16, tag="recip")
                prev_recip = nc.scalar.activation(
                    recip[:, 0:cw], lncs[:, 0:cw], AF.Exp, scale=-1.0
                )
                rep_ps = ps_m.tile([P, CH], f32, tag="misc")
                nc.tensor.matmul(
                    rep_ps[:, 0:cw], ones_row[:, :], recip[:, 0:cw],
                    start=True, stop=True,
                )
                rep = smallp.tile([P, CH], bf16, tag="rep")
                nc.vector.tensor_copy(rep[:, 0:cw], rep_ps[:, 0:cw])

                w = wp.tile([P, T * CH], bf16, tag="w")
                w3 = w[:].rearrange("p (t n) -> p t n", t=T)[:, :, 0:cw]
                w4 = w[:].rearrange("p (g two n) -> p g two n", g=NP, two=2)[
                    :, :, :, 0:cw
                ]
                u3c = u3
                for h in range(2):
                    tsl = slice(h * HALF, (h + 1) * HALF)
                    gsl = slice(h * (NP // 2), (h + 1) * (NP // 2))
                    rep_b = rep[:, 0:cw].unsqueeze(1).broadcast_to([P, HALF, cw])
                    nc.vector.tensor_mul(w3[:, tsl, :], u3c[:, tsl, :], rep_b)
                    for jj in range(2):
                        nc.gpsimd.dma_start(
                            wo4[:, gsl, jj, o0 : o0 + cw], w4[:, gsl, jj, :]
                        )

                # stash unnormalized vals; normalize one chunk behind so the
                # small ACT/PE/DVE ops slot into idle space off the DMA path
                nc.vector.tensor_copy(vals_all[:, qsl], vals_ps[0:V, :])
                filler_mm(3)
                if c >= 1:
                    values_block(CHUNKS[c - 1])
            values_block(CHUNKS[-1])

            # keep the verifier happy: PSUM locations need a reader
            nc.vector.tensor_copy(wu_src[0:1, 0:1], wu_ps[0:1, 0:1])

    nc.compile()
    return nc


def get_nc():
    if "nc" not in _CACHE:
        _CACHE["nc"] = _build()
    return _CACHE["nc"]


def kernel(key, query, value):
    from concourse.bass_utils import run_bass_kernel_spmd

    key = np.ascontiguousarray(np.asarray(key, dtype=np.float32))
    query = np.ascontiguousarray(np.asarray(query, dtype=np.float32))
    value = np.ascontiguousarray(np.asarray(value, dtype=np.float32))

    nc = get_nc()
    in_maps = [
        {
            "key": np.ascontiguousarray(key[i, :, 0, :]),
            "query": np.ascontiguousarray(query[i, 0]),
            "value": np.ascontiguousarray(value[i]),
        }
        for i in range(B)
    ]
    res = run_bass_kernel_spmd(nc, in_maps, core_ids=list(range(B)))
    values = np.stack([res.results[i]["values_out"] for i in range(B)])
    weights = np.stack([res.results[i]["weights_out"] for i in range(B)])
    return values, weights


# revision 37
# speedup vs baseline: 1.1314x; 1.1314x over previous
"""Trainium2 Bass kernel for nn_AttentionLayer (sparse_attention).

reference:
    S[i,o]   = sum_k key[b,i,0,k] * query[b,0,o,k] / sqrt(64)
    W        = softmax(S, axis=i)                      # weights output [B, NI, NO]
    vals     = einsum("bio,biv->bov", W, value)
    vals     = vals / (||vals||_2(axis=v) + 1e-12)
    values   = swapaxes(vals, 1, 2)                    # [B, V, NO]
    returns (values, W)

Sharding: data-parallel over batch, one batch element per NeuronCore (B=8).

Per-core pipeline (bf16 matmuls, f32 accumulation):
  - inputs cast to bf16 on-chip; K and V use a paired-row load (two DRAM rows
    per partition -> 512B descriptors at line rate), which permutes the i
    order; the weights-output DMA access pattern compensates.
  - KT2: K^T with k on partitions, even/odd-row tiles stacked in the two
    64-partition halves so the QK^T matmuls run two i-tiles concurrently via
    PE row tiling; QT2 = Q^T duplicated into both halves.
  - per o-chunk (4 x 512):
      per i-tile pair (8): S = 2 concurrent matmuls (K=64) -> [128, 2x512]
        PSUM (separate banks); U = exp(0.125*S) -> bf16 (one ACT op/pair);
        vals_ps += matmul(lhsT=V'[t], rhs=U[t])  ([65, 512]; the V' ones
        column makes row 64 the softmax denominator)
      recip = exp(-ln(colsum)) on ACT (stays in the exp table set);
      rep = ones-matmul broadcast of recip; W = U * rep (DVE bf16 2x) ->
      SWDGE DMA with bf16->f32 cast
      values normalization (rsqrt via exp(-0.5 ln)) runs one chunk behind,
      off the critical path; softmax denominators cancel inside the L2 norm.
  - dummy filler matmuls keep the PE activity monitor from re-throttling the
    clock to 1.2 GHz during ACT/DMA-bound stretches.
"""

import numpy as np

B, NI, NO, K, V = 8, 2048, 2048, 64, 64
P = 128          # partitions / i-tile height
T = NI // P      # 16 i-tiles
NP = T // 2      # 8 i-tile pairs
CH = 512         # o-chunk width (one PSUM bank of f32)
NCH = NO // CH   # 4 chunks
HALF = T // 2    # W scale/DMA granularity (half a chunk)
WARMUP_MM = 10

_CACHE = {}


def _build():
    import concourse.bass as bass
    import concourse.tile as tile
    from concourse import bacc, mybir
    from concourse.bass import _add_dep_helper
    from concourse.masks import make_identity

    f32 = mybir.dt.float32
    bf16 = mybir.dt.bfloat16
    AF = mybir.ActivationFunctionType

    nc = bacc.Bacc("TRN2", target_bir_lowering=False, debug=False, num_devices=8)
    key_d = nc.declare_dram_parameter("key", [NI, K], f32, isOutput=False)
    query_d = nc.declare_dram_parameter("query", [NO, K], f32, isOutput=False)
    value_d = nc.declare_dram_parameter("value", [NI, V], f32, isOutput=False)
    values_o = nc.declare_dram_parameter("values_out", [V, NO], f32, isOutput=True)
    weights_o = nc.declare_dram_parameter("weights_out", [NI, NO], f32, isOutput=True)

    with tile.TileContext(nc) as tc:
        with (
            tc.tile_pool(name="const", bufs=1) as constp,
            tc.tile_pool(name="stage", bufs=2) as stagep,
            tc.tile_pool(name="u", bufs=3) as up,
            tc.tile_pool(name="w", bufs=3) as wp,
            tc.tile_pool(name="small", bufs=3) as smallp,
            tc.tile_pool(name="ps_s", bufs=2, space="PSUM") as ps_s,
            tc.tile_pool(name="ps_v", bufs=2, space="PSUM") as ps_v,
            tc.tile_pool(name="ps_m", bufs=2, space="PSUM") as ps_m,
        ):
            # Pin the ACT spline table set to one containing Exp+Ln+Copy so the
            # compiler never has to switch sets mid-kernel (~2.7us per switch).
            try:
                from concourse.hw_specs import get_activation_tables

                tabs = get_activation_tables(nc.m.arch)
                want = {AF.Exp, AF.Ln, AF.Copy}
                set_id = next(
                    i for i, fns in enumerate(tabs.values()) if want <= fns
                )
                nc.scalar.add_instruction(
                    mybir.InstLoadActFuncSet(
                        name=nc.get_next_instruction_name(),
                        act_func_set_id=set_id,
                        ins=[],
                        outs=[],
                    )
                )
            except Exception:
                pass  # fall back to compiler-inserted table loads

            ident = constp.tile([P, P], f32)
            make_identity(nc, ident[:])
            ones_row = constp.tile([1, P], bf16)
            nc.vector.memset(ones_row[:], 1.0)
            ones_col = constp.tile([K, 1], bf16)
            nc.vector.memset(ones_col[:], 1.0)

            # wu_ps: dummy-matmul target; filler matmuls keep the PE HAM warm
            wu_src = constp.tile([K, CH], bf16)
            nc.vector.memset(wu_src[:], 0.0)
            wu_ps = ps_m.tile([P, CH], f32, tag="misc")

            def filler_mm(n=1):
                for _ in range(n):
                    nc.tensor.matmul(
                        wu_ps[:], wu_src[:, 0:P], wu_src[:], start=True, stop=True
                    )

            # K^T/Q^T bf16, duplicated across both 64-partition halves:
            #   KT2[0:64, pt*128:+128]  = K^T of even rows of pair-group pt
            #   KT2[64:128, pt*128:+128]= K^T of odd rows
            #   QT2[0:64, :] = Q^T ; QT2[64:128, :] = copy of Q^T
            kt2 = constp.tile([P, NP * P], bf16)
            qt2 = constp.tile([P, NO], bf16)

            qstg_f = stagep.tile([P, T * K], f32, tag="qstg_f")
            kstg_f = stagep.tile([P, T * K], f32, tag="kstg_f")
            kstg_f3 = kstg_f[:].rearrange("p (g f) -> p g f", g=NP)
            qstg_f3 = qstg_f[:].rearrange("p (t k) -> p t k", t=T)
            kv = key_d.ap().rearrange("(g p two) k -> p g (two k)", g=NP, p=P)
            qv = query_d.ap().rearrange("(t p) k -> p t k", p=P)
            HT = T // 2
            for qq in range(4):
                nc.sync.dma_start(
                    kstg_f3[:, 2 * qq : 2 * qq + 2, :],
                    kv[:, 2 * qq : 2 * qq + 2, :],
                )
            for hh in range(2):
                nc.scalar.dma_start(
                    qstg_f3[:, hh * HT : (hh + 1) * HT, :],
                    qv[:, hh * HT : (hh + 1) * HT, :],
                )

            # V' = [value | ones] bf16, per i-tile: [128, 65]
            vp = constp.tile([P, T * (V + 1)], bf16)
            vp3 = vp[:].rearrange("p (t c) -> p t c", t=T)
            nc.vector.memset(vp3[:, :, V : V + 1], 1.0)
            vstg = stagep.tile([P, T * V], f32, tag="vstg")
            nc.scalar.dma_start(
                vstg[:].rearrange("p (g f) -> p g f", g=NP),
                value_d.ap().rearrange("(g p two) v -> p g (two v)", g=NP, p=P),
            )

            def q_group(g):  # Q tiles 4g..4g+3 -> QT2 cols [512g, 512g+512)
                tp = ps_m.tile([K, 4 * P], f32, tag="misc")
                for j in range(4):
                    nc.tensor.transpose(
                        tp[:, j * P : (j + 1) * P], qstg_f3[:, 4 * g + j, :], ident[:]
                    )
                csl = slice(g * 4 * P, (g + 1) * 4 * P)
                nc.vector.tensor_copy(qt2[0:K, csl], tp[:])
                nc.vector.tensor_copy(qt2[K:P, csl], tp[:])

            def k_pair(pt):  # paired group pt -> KT2 col block pt (both halves)
                tp = ps_m.tile([P, P], f32, tag="misc")
                nc.tensor.transpose(tp[:], kstg_f3[:, pt, :], ident[:])
                nc.vector.tensor_copy(kt2[:, pt * P : (pt + 1) * P], tp[:])

            k_pair(0)
            k_pair(1)
            q_group(0)
            for qq in range(1, 4):
                k_pair(2 * qq)
                k_pair(2 * qq + 1)
            q_group(1)
            nc.vector.tensor_copy(
                vp3[:, :, 0:V], vstg[:].rearrange("p (t v) -> p t v", t=T)
            )
            q_group(2)
            q_group(3)

            vals_all = constp.tile([V, NO], f32)
            wo4 = weights_o.ap().rearrange("(g p two) o -> p g two o", g=NP, p=P)

            def values_block(span):
                # values = vals * rsqrt(sum_v vals^2); softmax denoms cancel
                o0b, cwb = span
                csl = slice(o0b, o0b + cwb)
                sq = smallp.tile([V, CH], bf16, tag="sq")
                nc.vector.tensor_mul(
                    sq[:, 0:cwb], vals_all[:, csl], vals_all[:, csl]
                )
                ss_ps = ps_m.tile([1, CH], f32, tag="misc")
                nc.tensor.matmul(
                    ss_ps[:, 0:cwb], ones_col[:, :], sq[:, 0:cwb],
                    start=True, stop=True,
                )
                lnt = smallp.tile([1, CH], f32, tag="lnt")
                nc.scalar.activation(lnt[:, 0:cwb], ss_ps[:, 0:cwb], AF.Ln)
                invn = smallp.tile([1, CH], bf16, tag="invn")
                nc.scalar.activation(
                    invn[:, 0:cwb], lnt[:, 0:cwb], AF.Exp, scale=-0.5
                )
                repn_ps = ps_m.tile([V, CH], f32, tag="misc")
                nc.tensor.matmul(
                    repn_ps[:, 0:cwb], ones_row[:, 0:V], invn[:, 0:cwb],
                    start=True, stop=True,
                )
                vouts = smallp.tile([V, CH], f32, tag="vouts")
                nc.vector.tensor_mul(
                    vouts[:, 0:cwb], vals_all[:, csl], repn_ps[:, 0:cwb]
                )
                nc.sync.dma_start(values_o.ap()[:, csl], vouts[:, 0:cwb])

            # chunk plan: narrow leading chunks so the weights DMA starts early
            CHUNKS = [(0, 512), (512, 512), (1024, 512), (1536, 512)]
            prev_recip = None
            for c, (o0, cw) in enumerate(CHUNKS):
                u = up.tile([P, T * cw], bf16, tag="u")
                u3 = u[:].rearrange("p (t n) -> p t n", t=T)
                vals_ps = ps_v.tile([V + 1, cw], f32, tag="vals")
                qsl = slice(o0, o0 + cw)
                for pt in range(NP):
                    # concurrent row-tiled matmuls must land in different PSUM
                    # banks: pin halves at CH-sized bank offsets even when
                    # cw < CH
                    s_ps = ps_s.tile([P, 2 * CH], f32, tag="s")
                    for j in range(2):
                        nc.tensor.matmul(
                            s_ps[:, j * CH : j * CH + cw],
                            kt2[j * K : (j + 1) * K, pt * P : (pt + 1) * P],
                            qt2[j * K : (j + 1) * K, qsl],
                            start=True,
                            stop=True,
                        )
                    exp_inst = nc.scalar.activation(
                        u3[:, 2 * pt : 2 * pt + 2, :],
                        s_ps[:].rearrange("p (j n) -> p j n", j=2)[:, :, 0:cw],
                        AF.Exp,
                        scale=0.125,
                    )
                    if pt == 0 and prev_recip is not None:
                        # keep the previous chunk's weights-DMA-gating recip
                        # ahead of this chunk's exps in the ACT stream
                        _add_dep_helper(
                            exp_inst.ins, prev_recip.ins,
                            sync=False, reason="recip before next-chunk exps",
                        )
                    for j in range(2):
                        t = 2 * pt + j
                        nc.tensor.matmul(
                            vals_ps[:],
                            vp3[:, t, :],
                            u3[:, t, :],
                            start=(t == 0),
                            stop=(t == T - 1),
                        )
                    if c >= 1 and pt % 2 == 1:
                        filler_mm()

                # weights = U * (1/colsum); recip via exp(-ln) stays in the exp
                # table set (ACT Reciprocal banned; DVE recip on 1 partition is
                # 3.4us). High priority: this chain gates the weights DMA, so
                # it must beat the next chunk's exps to the ACT queue.
                lncs = smallp.tile([1, CH], f32, tag="lncs")
                nc.scalar.activation(lncs[:, 0:cw], vals_ps[V : V + 1, :], AF.Ln)
                recip = smallp.tile([1, CH], bf16, tag="recip")
                prev_recip = nc.scalar.activation(
                    recip[:, 0:cw], lncs[:, 0:cw], AF.Exp, scale=-1.0
                )
                rep_ps = ps_m.tile([P, CH], f32, tag="misc")
                nc.tensor.matmul(
                    rep_ps[:, 0:cw], ones_row[:, :], recip[:, 0:cw],
                    start=True, stop=True,
                )
                rep = smallp.tile([P, CH], bf16, tag="rep")
                nc.vector.tensor_copy(rep[:, 0:cw], rep_ps[:, 0:cw])

                w = wp.tile([P, T * CH], bf16, tag="w")
                w3 = w[:].rearrange("p (t n) -> p t n", t=T)[:, :, 0:cw]
                w4 = w[:].rearrange("p (g two n) -> p g two n", g=NP, two=2)[
                    :, :, :, 0:cw
                ]
                u3c = u3
                for h in range(2):
                    tsl = slice(h * HALF, (h + 1) * HALF)
                    gsl = slice(h * (NP // 2), (h + 1) * (NP // 2))
                    rep_b = rep[:, 0:cw].unsqueeze(1).broadcast_to([P, HALF, cw])
                    nc.vector.tensor_mul(w3[:, tsl, :], u3c[:, tsl, :], rep_b)
                    for jj in range(2):
                        nc.gpsimd.dma_start(
                            wo4[:, gsl, jj, o0 : o0 + cw], w4[:, gsl, jj, :]
                        )

                # stash unnormalized vals; normalize one chunk behind so the
                # small ACT/PE/DVE ops slot into idle space off the DMA path
                nc.vector.tensor_copy(vals_all[:, qsl], vals_ps[0:V, :])
                filler_mm(3)
                if c >= 1:
                    values_block(CHUNKS[c - 1])
            values_block(CHUNKS[-1])

            # keep the verifier happy: PSUM locations need a reader
            nc.vector.tensor_copy(wu_src[0:1, 0:1], wu_ps[0:1, 0:1])

    nc.compile()
    return nc


def get_nc():
    if "nc" not in _CACHE:
        _CACHE["nc"] = _build()
    return _CACHE["nc"]


def kernel(key, query, value):
    from concourse.bass_utils import run_bass_kernel_spmd

    key = np.ascontiguousarray(np.asarray(key, dtype=np.float32))
    query = np.ascontiguousarray(np.asarray(query, dtype=np.float32))
    value = np.ascontiguousarray(np.asarray(value, dtype=np.float32))

    nc = get_nc()
    in_maps = [
        {
            "key": np.ascontiguousarray(key[i, :, 0, :]),
            "query": np.ascontiguousarray(query[i, 0]),
            "value": np.ascontiguousarray(value[i]),
        }
        for i in range(B)
    ]
    res = run_bass_kernel_spmd(nc, in_maps, core_ids=list(range(B)))
    values = np.stack([res.results[i]["values_out"] for i in range(B)])
    weights = np.stack([res.results[i]["weights_out"] for i in range(B)])
    return values, weights


# revision 38
# speedup vs baseline: 1.2346x; 1.0913x over previous
"""Trainium2 Bass kernel for nn_AttentionLayer (sparse_attention).

reference:
    S[i,o]   = sum_k key[b,i,0,k] * query[b,0,o,k] / sqrt(64)
    W        = softmax(S, axis=i)                      # weights output [B, NI, NO]
    vals     = einsum("bio,biv->bov", W, value)
    vals     = vals / (||vals||_2(axis=v) + 1e-12)
    values   = swapaxes(vals, 1, 2)                    # [B, V, NO]
    returns (values, W)

Sharding: data-parallel over batch, one batch element per NeuronCore (B=8).

Per-core pipeline (bf16 matmuls, f32 accumulation):
  - inputs cast to bf16 on-chip; K and V use a paired-row load (two DRAM rows
    per partition -> 512B descriptors at line rate), which permutes the i
    order; the weights-output DMA access pattern compensates.
  - KT2: K^T with k on partitions, even/odd-row tiles stacked in the two
    64-partition halves so the QK^T matmuls run two i-tiles concurrently via
    PE row tiling; QT2 = Q^T duplicated into both halves.
  - per o-chunk (4 x 512):
      per i-tile pair (8): S = 2 concurrent matmuls (K=64) -> [128, 2x512]
        PSUM (separate banks); U = exp(0.125*S) -> bf16 (one ACT op/pair);
        vals_ps += matmul(lhsT=V'[t], rhs=U[t])  ([65, 512]; the V' ones
        column makes row 64 the softmax denominator)
      recip = exp(-ln(colsum)) on ACT (stays in the exp table set);
      rep = ones-matmul broadcast of recip; W = U * rep (DVE bf16 2x) ->
      SWDGE DMA with bf16->f32 cast
      values normalization (rsqrt via exp(-0.5 ln)) runs one chunk behind,
      off the critical path; softmax denominators cancel inside the L2 norm.
  - dummy filler matmuls keep the PE activity monitor from re-throttling the
    clock to 1.2 GHz during ACT/DMA-bound stretches.
"""

import numpy as np

B, NI, NO, K, V = 8, 2048, 2048, 64, 64
P = 128          # partitions / i-tile height
T = NI // P      # 16 i-tiles
NP = T // 2      # 8 i-tile pairs
CH = 512         # o-chunk width (one PSUM bank of f32)
NCH = NO // CH   # 4 chunks
HALF = T // 2    # W scale/DMA granularity (half a chunk)

_CACHE = {}


def _build():
    import concourse.bass as bass
    import concourse.tile as tile
    from concourse import bacc, mybir
    from concourse.bass import _add_dep_helper
    from concourse.masks import make_identity

    f32 = mybir.dt.float32
    bf16 = mybir.dt.bfloat16
    AF = mybir.ActivationFunctionType

    nc = bacc.Bacc("TRN2", target_bir_lowering=False, debug=False, num_devices=8)
    key_d = nc.declare_dram_parameter("key", [NI, K], f32, isOutput=False)
    query_d = nc.declare_dram_parameter("query", [NO, K], f32, isOutput=False)
    value_d = nc.declare_dram_parameter("value", [NI, V], f32, isOutput=False)
    values_o = nc.declare_dram_parameter("values_out", [V, NO], f32, isOutput=True)
    weights_o = nc.declare_dram_parameter("weights_out", [NI, NO], f32, isOutput=True)

    with tile.TileContext(nc) as tc:
        with (
            tc.tile_pool(name="const", bufs=1) as constp,
            tc.tile_pool(name="stage", bufs=2) as stagep,
            tc.tile_pool(name="u", bufs=3) as up,
            tc.tile_pool(name="w", bufs=3) as wp,
            tc.tile_pool(name="small", bufs=3) as smallp,
            tc.tile_pool(name="ps_s", bufs=2, space="PSUM") as ps_s,
            tc.tile_pool(name="ps_v", bufs=2, space="PSUM") as ps_v,
            tc.tile_pool(name="ps_m", bufs=2, space="PSUM") as ps_m,
        ):
            # Pin the ACT spline table set to one containing Exp+Ln+Copy so the
            # compiler never has to switch sets mid-kernel (~2.7us per switch).
            try:
                from concourse.hw_specs import get_activation_tables

                tabs = get_activation_tables(nc.m.arch)
                want = {AF.Exp, AF.Ln, AF.Copy}
                set_id = next(
                    i for i, fns in enumerate(tabs.values()) if want <= fns
                )
                nc.scalar.add_instruction(
                    mybir.InstLoadActFuncSet(
                        name=nc.get_next_instruction_name(),
                        act_func_set_id=set_id,
                        ins=[],
                        outs=[],
                    )
                )
            except Exception:
                pass  # fall back to compiler-inserted table loads

            ident = constp.tile([P, P], f32)
            make_identity(nc, ident[:])
            ones_row = constp.tile([1, P], bf16)
            nc.vector.memset(ones_row[:], 1.0)
            ones_col = constp.tile([K, 1], bf16)
            nc.vector.memset(ones_col[:], 1.0)

            # wu_ps: dummy-matmul target; filler matmuls keep the PE HAM warm
            wu_src = constp.tile([K, CH], bf16)
            nc.vector.memset(wu_src[:], 0.0)
            wu_ps = ps_m.tile([P, CH], f32, tag="misc")

            def filler_mm(n=1):
                for _ in range(n):
                    nc.tensor.matmul(
                        wu_ps[:], wu_src[:, 0:P], wu_src[:], start=True, stop=True
                    )

            # K^T/Q^T bf16, duplicated across both 64-partition halves:
            #   KT2[0:64, pt*128:+128]  = K^T of even rows of pair-group pt
            #   KT2[64:128, pt*128:+128]= K^T of odd rows
            #   QT2[0:64, :] = Q^T ; QT2[64:128, :] = copy of Q^T
            kt2 = constp.tile([P, NP * P], bf16)
            qt2 = constp.tile([P, NO], bf16)

            qstg_f = stagep.tile([P, T * K], f32, tag="qstg_f")
            kstg_f = stagep.tile([P, T * K], f32, tag="kstg_f")
            kstg_f3 = kstg_f[:].rearrange("p (g f) -> p g f", g=NP)
            qstg_f3 = qstg_f[:].rearrange("p (t k) -> p t k", t=T)
            kv = key_d.ap().rearrange("(g p two) k -> p g (two k)", g=NP, p=P)
            qv = query_d.ap().rearrange("(t p) k -> p t k", p=P)
            HT = T // 2
            for qq in range(4):
                nc.sync.dma_start(
                    kstg_f3[:, 2 * qq : 2 * qq + 2, :],
                    kv[:, 2 * qq : 2 * qq + 2, :],
                )
            for hh in range(2):
                nc.scalar.dma_start(
                    qstg_f3[:, hh * HT : (hh + 1) * HT, :],
                    qv[:, hh * HT : (hh + 1) * HT, :],
                )

            # V' = [value | ones] bf16, per i-tile: [128, 65]
            vp = constp.tile([P, T * (V + 1)], bf16)
            vp3 = vp[:].rearrange("p (t c) -> p t c", t=T)
            nc.vector.memset(vp3[:, :, V : V + 1], 1.0)
            vstg = stagep.tile([P, T * V], f32, tag="vstg")
            nc.scalar.dma_start(
                vstg[:].rearrange("p (g f) -> p g f", g=NP),
                value_d.ap().rearrange("(g p two) v -> p g (two v)", g=NP, p=P),
            )

            def q_group(g):  # Q tiles 4g..4g+3 -> QT2 cols [512g, 512g+512)
                tp = ps_m.tile([K, 4 * P], f32, tag="misc")
                for j in range(4):
                    nc.tensor.transpose(
                        tp[:, j * P : (j + 1) * P], qstg_f3[:, 4 * g + j, :], ident[:]
                    )
                csl = slice(g * 4 * P, (g + 1) * 4 * P)
                nc.vector.tensor_copy(qt2[0:K, csl], tp[:])
                nc.vector.tensor_copy(qt2[K:P, csl], tp[:])

            def k_pair(pt):  # paired group pt -> KT2 col block pt (both halves)
                tp = ps_m.tile([P, P], f32, tag="misc")
                nc.tensor.transpose(tp[:], kstg_f3[:, pt, :], ident[:])
                nc.vector.tensor_copy(kt2[:, pt * P : (pt + 1) * P], tp[:])

            k_pair(0)
            k_pair(1)
            q_group(0)
            for qq in range(1, 4):
                k_pair(2 * qq)
                k_pair(2 * qq + 1)
            q_group(1)
            nc.vector.tensor_copy(
                vp3[:, :, 0:V], vstg[:].rearrange("p (t v) -> p t v", t=T)
            )
            q_group(2)
            q_group(3)

            vals_all = constp.tile([V, NO], f32)
            wo4 = weights_o.ap().rearrange("(g p two) o -> p g two o", g=NP, p=P)

            def values_block(span):
                # values = vals * rsqrt(sum_v vals^2); softmax denoms cancel
                o0b, cwb = span
                csl = slice(o0b, o0b + cwb)
                sq = smallp.tile([V, CH], bf16, tag="sq")
                nc.vector.tensor_mul(
                    sq[:, 0:cwb], vals_all[:, csl], vals_all[:, csl]
                )
                ss_ps = ps_m.tile([1, CH], f32, tag="misc")
                nc.tensor.matmul(
                    ss_ps[:, 0:cwb], ones_col[:, :], sq[:, 0:cwb],
                    start=True, stop=True,
                )
                lnt = smallp.tile([1, CH], f32, tag="lnt")
                nc.scalar.activation(lnt[:, 0:cwb], ss_ps[:, 0:cwb], AF.Ln)
                invn = smallp.tile([1, CH], bf16, tag="invn")
                nc.scalar.activation(
                    invn[:, 0:cwb], lnt[:, 0:cwb], AF.Exp, scale=-0.5
                )
                repn_ps = ps_m.tile([V, CH], f32, tag="misc")
                nc.tensor.matmul(
                    repn_ps[:, 0:cwb], ones_row[:, 0:V], invn[:, 0:cwb],
                    start=True, stop=True,
                )
                vouts = smallp.tile([V, CH], f32, tag="vouts")
                nc.vector.tensor_mul(
                    vouts[:, 0:cwb], vals_all[:, csl], repn_ps[:, 0:cwb]
                )
                nc.sync.dma_start(values_o.ap()[:, csl], vouts[:, 0:cwb])

            # chunk plan: narrow leading chunks so the weights DMA starts early
            CHUNKS = [(0, 512), (512, 512), (1024, 512), (1536, 512)]
            prev_recip = None
            for c, (o0, cw) in enumerate(CHUNKS):
                u = up.tile([P, T * cw], bf16, tag="u")
                u3 = u[:].rearrange("p (t n) -> p t n", t=T)
                vals_ps = ps_v.tile([V + 1, cw], f32, tag="vals")
                qsl = slice(o0, o0 + cw)
                for pt in range(NP):
                    # concurrent row-tiled matmuls must land in different PSUM
                    # banks: pin halves at CH-sized bank offsets even when
                    # cw < CH
                    s_ps = ps_s.tile([P, 2 * CH], f32, tag="s")
                    for j in range(2):
                        nc.tensor.matmul(
                            s_ps[:, j * CH : j * CH + cw],
                            kt2[j * K : (j + 1) * K, pt * P : (pt + 1) * P],
                            qt2[j * K : (j + 1) * K, qsl],
                            start=True,
                            stop=True,
                        )
                    exp_inst = nc.scalar.activation(
                        u3[:, 2 * pt : 2 * pt + 2, :],
                        s_ps[:].rearrange("p (j n) -> p j n", j=2)[:, :, 0:cw],
                        AF.Exp,
                        scale=0.125,
                    )
                    if pt == 0 and prev_recip is not None:
                        # keep the previous chunk's weights-DMA-gating recip
                        # ahead of this chunk's exps in the ACT stream
                        _add_dep_helper(
                            exp_inst.ins, prev_recip.ins,
                            sync=False, reason="recip before next-chunk exps",
                        )
                    for j in range(2):
                        t = 2 * pt + j
                        nc.tensor.matmul(
                            vals_ps[:],
                            vp3[:, t, :],
                            u3[:, t, :],
                            start=(t == 0),
                            stop=(t == T - 1),
                        )
                    if c >= 1 and pt % 2 == 1:
                        filler_mm()

                # weights = U * (1/colsum); recip via exp(-ln) stays in the exp
                # table set (ACT Reciprocal banned; DVE recip on 1 partition is
                # 3.4us). High priority: this chain gates the weights DMA, so
                # it must beat the next chunk's exps to the ACT queue.
                lncs = smallp.tile([1, CH], f32, tag="lncs")
                nc.scalar.activation(lncs[:, 0:cw], vals_ps[V : V + 1, :], AF.Ln)
                recip = smallp.tile([1, CH], bf16, tag="recip")
                prev_recip = nc.scalar.activation(
                    recip[:, 0:cw], lncs[:, 0:cw], AF.Exp, scale=-1.0
                )
                rep_ps = ps_m.tile([P, CH], f32, tag="misc")
                nc.tensor.matmul(
                    rep_ps[:, 0:cw], ones_row[:, :], recip[:, 0:cw],
                    start=True, stop=True,
                )
                rep = smallp.tile([P, CH], bf16, tag="rep")
                nc.vector.tensor_copy(rep[:, 0:cw], rep_ps[:, 0:cw])

                w = wp.tile([P, T * CH], bf16, tag="w")
                w3 = w[:].rearrange("p (t n) -> p t n", t=T)[:, :, 0:cw]
                w4 = w[:].rearrange("p (g two n) -> p g two n", g=NP, two=2)[
                    :, :, :, 0:cw
                ]
                u3c = u3
                for h in range(2):
                    tsl = slice(h * HALF, (h + 1) * HALF)
                    gsl = slice(h * (NP // 2), (h + 1) * (NP // 2))
                    rep_b = rep[:, 0:cw].unsqueeze(1).broadcast_to([P, HALF, cw])
                    nc.vector.tensor_mul(w3[:, tsl, :], u3c[:, tsl, :], rep_b)
                    for jj in range(2):
                        nc.gpsimd.dma_start(
                            wo4[:, gsl, jj, o0 : o0 + cw], w4[:, gsl, jj, :]
                        )

                # stash unnormalized vals; normalize one chunk behind so the
                # small ACT/PE/DVE ops slot into idle space off the DMA path
                nc.vector.tensor_copy(vals_all[:, qsl], vals_ps[0:V, :])
                filler_mm(3)
                if c >= 1:
                    values_block(CHUNKS[c - 1])
            values_block(CHUNKS[-1])

            # keep the verifier happy: PSUM locations need a reader
            nc.vector.tensor_copy(wu_src[0:1, 0:1], wu_ps[0:1, 0:1])

    nc.compile()
    return nc


def get_nc():
    if "nc" not in _CACHE:
        _CACHE["nc"] = _build()
    return _CACHE["nc"]


def kernel(key, query, value):
    from concourse.bass_utils import run_bass_kernel_spmd

    key = np.ascontiguousarray(np.asarray(key, dtype=np.float32))
    query = np.ascontiguousarray(np.asarray(query, dtype=np.float32))
    value = np.ascontiguousarray(np.asarray(value, dtype=np.float32))

    nc = get_nc()
    in_maps = [
        {
            "key": np.ascontiguousarray(key[i, :, 0, :]),
            "query": np.ascontiguousarray(query[i, 0]),
            "value": np.ascontiguousarray(value[i]),
        }
        for i in range(B)
    ]
    res = run_bass_kernel_spmd(nc, in_maps, core_ids=list(range(B)))
    values = np.stack([res.results[i]["values_out"] for i in range(B)])
    weights = np.stack([res.results[i]["weights_out"] for i in range(B)])
    return values, weights


# revision 39
# speedup vs baseline: 1.3086x; 1.0600x over previous
"""Trainium2 Bass kernel for nn_AttentionLayer (sparse_attention).

reference:
    S[i,o]   = sum_k key[b,i,0,k] * query[b,0,o,k] / sqrt(64)
    W        = softmax(S, axis=i)                      # weights output [B, NI, NO]
    vals     = einsum("bio,biv->bov", W, value)
    vals     = vals / (||vals||_2(axis=v) + 1e-12)
    values   = swapaxes(vals, 1, 2)                    # [B, V, NO]
    returns (values, W)

Sharding: data-parallel over batch, one batch element per NeuronCore (B=8).

Per-core pipeline (bf16 matmuls, f32 accumulation):
  - inputs cast to bf16 on-chip; K and V use a paired-row load (two DRAM rows
    per partition -> 512B descriptors at line rate), which permutes the i
    order; the weights-output DMA access pattern compensates.
  - KT2: K^T with k on partitions, even/odd-row tiles stacked in the two
    64-partition halves so the QK^T matmuls run two i-tiles concurrently via
    PE row tiling; QT2 = Q^T duplicated into both halves.
  - per o-chunk (4 x 512):
      per i-tile pair (8): S = 2 concurrent matmuls (K=64) -> [128, 2x512]
        PSUM (separate banks); U = exp(0.125*S) -> bf16 (one ACT op/pair);
        vals_ps += matmul(lhsT=V'[t], rhs=U[t])  ([65, 512]; the V' ones
        column makes row 64 the softmax denominator)
      recip = exp(-ln(colsum)) on ACT (stays in the exp table set);
      rep = ones-matmul broadcast of recip; W = U * rep (DVE bf16 2x) ->
      SWDGE DMA with bf16->f32 cast
      values normalization (rsqrt via exp(-0.5 ln)) runs one chunk behind,
      off the critical path; softmax denominators cancel inside the L2 norm.
  - dummy filler matmuls keep the PE activity monitor from re-throttling the
    clock to 1.2 GHz during ACT/DMA-bound stretches.
"""

import numpy as np

B, NI, NO, K, V = 8, 2048, 2048, 64, 64
P = 128          # partitions / i-tile height
T = NI // P      # 16 i-tiles
NP = T // 2      # 8 i-tile pairs
CH = 512         # o-chunk width (one PSUM bank of f32)
NCH = NO // CH   # 4 chunks
HALF = T // 2    # W scale/DMA granularity (half a chunk)

_CACHE = {}


def _build():
    import concourse.bass as bass
    import concourse.tile as tile
    from concourse import bacc, mybir
    from concourse.bass import _add_dep_helper
    from concourse.masks import make_identity

    f32 = mybir.dt.float32
    bf16 = mybir.dt.bfloat16
    AF = mybir.ActivationFunctionType

    nc = bacc.Bacc("TRN2", target_bir_lowering=False, debug=False, num_devices=8)
    key_d = nc.declare_dram_parameter("key", [NI, K], f32, isOutput=False)
    query_d = nc.declare_dram_parameter("query", [NO, K], f32, isOutput=False)
    value_d = nc.declare_dram_parameter("value", [NI, V], f32, isOutput=False)
    values_o = nc.declare_dram_parameter("values_out", [V, NO], f32, isOutput=True)
    weights_o = nc.declare_dram_parameter("weights_out", [NI, NO], f32, isOutput=True)

    with tile.TileContext(nc) as tc:
        with (
            tc.tile_pool(name="const", bufs=1) as constp,
            tc.tile_pool(name="stage", bufs=2) as stagep,
            tc.tile_pool(name="u", bufs=3) as up,
            tc.tile_pool(name="w", bufs=3) as wp,
            tc.tile_pool(name="small", bufs=3) as smallp,
            tc.tile_pool(name="ps_s", bufs=2, space="PSUM") as ps_s,
            tc.tile_pool(name="ps_v", bufs=2, space="PSUM") as ps_v,
            tc.tile_pool(name="ps_m", bufs=2, space="PSUM") as ps_m,
        ):
            # Pin the ACT spline table set to one containing Exp+Ln+Copy so the
            # compiler never has to switch sets mid-kernel (~2.7us per switch).
            try:
                from concourse.hw_specs import get_activation_tables

                tabs = get_activation_tables(nc.m.arch)
                want = {AF.Exp, AF.Ln, AF.Copy}
                set_id = next(
                    i for i, fns in enumerate(tabs.values()) if want <= fns
                )
                nc.scalar.add_instruction(
                    mybir.InstLoadActFuncSet(
                        name=nc.get_next_instruction_name(),
                        act_func_set_id=set_id,
                        ins=[],
                        outs=[],
                    )
                )
            except Exception:
                pass  # fall back to compiler-inserted table loads

            ident = constp.tile([P, P], f32)
            make_identity(nc, ident[:])
            ones_row = constp.tile([1, P], bf16)
            nc.vector.memset(ones_row[:], 1.0)
            ones_col = constp.tile([K, 1], bf16)
            nc.vector.memset(ones_col[:], 1.0)

            # wu_ps: dummy-matmul target; filler matmuls keep the PE HAM warm
            wu_src = constp.tile([K, CH], bf16)
            nc.vector.memset(wu_src[:], 0.0)
            wu_ps = ps_m.tile([P, CH], f32, tag="misc")

            def filler_mm(n=1):
                for _ in range(n):
                    nc.tensor.matmul(
                        wu_ps[:], wu_src[:, 0:P], wu_src[:], start=True, stop=True
                    )

            # K^T/Q^T bf16, duplicated across both 64-partition halves:
            #   KT2[0:64, pt*128:+128]  = K^T of even rows of pair-group pt
            #   KT2[64:128, pt*128:+128]= K^T of odd rows
            #   QT2[0:64, :] = Q^T ; QT2[64:128, :] = copy of Q^T
            kt2 = constp.tile([P, NP * P], bf16)
            qt2 = constp.tile([P, NO], bf16)

            qstg_f = stagep.tile([P, T * K], f32, tag="qstg_f")
            kstg_f = stagep.tile([P, T * K], f32, tag="kstg_f")
            kstg_f3 = kstg_f[:].rearrange("p (g f) -> p g f", g=NP)
            qstg_f3 = qstg_f[:].rearrange("p (t k) -> p t k", t=T)
            kv = key_d.ap().rearrange("(g p two) k -> p g (two k)", g=NP, p=P)
            qv = query_d.ap().rearrange("(t p) k -> p t k", p=P)
            HT = T // 2
            for qq in range(4):
                nc.sync.dma_start(
                    kstg_f3[:, 2 * qq : 2 * qq + 2, :],
                    kv[:, 2 * qq : 2 * qq + 2, :],
                )
            for hh in range(2):
                nc.scalar.dma_start(
                    qstg_f3[:, hh * HT : (hh + 1) * HT, :],
                    qv[:, hh * HT : (hh + 1) * HT, :],
                )

            # V' = [value | ones] bf16, per i-tile: [128, 65]
            vp = constp.tile([P, T * (V + 1)], bf16)
            vp3 = vp[:].rearrange("p (t c) -> p t c", t=T)
            nc.vector.memset(vp3[:, :, V : V + 1], 1.0)
            vstg = stagep.tile([P, T * V], f32, tag="vstg")
            nc.scalar.dma_start(
                vstg[:].rearrange("p (g f) -> p g f", g=NP),
                value_d.ap().rearrange("(g p two) v -> p g (two v)", g=NP, p=P),
            )

            def q_group(g):  # Q tiles 4g..4g+3 -> QT2 cols [512g, 512g+512)
                tp = ps_m.tile([K, 4 * P], f32, tag="misc")
                for j in range(4):
                    nc.tensor.transpose(
                        tp[:, j * P : (j + 1) * P], qstg_f3[:, 4 * g + j, :], ident[:]
                    )
                csl = slice(g * 4 * P, (g + 1) * 4 * P)
                nc.vector.tensor_copy(qt2[0:K, csl], tp[:])
                nc.vector.tensor_copy(qt2[K:P, csl], tp[:])

            def k_pair(pt):  # paired group pt -> KT2 col block pt (both halves)
                tp = ps_m.tile([P, P], f32, tag="misc")
                nc.tensor.transpose(tp[:], kstg_f3[:, pt, :], ident[:])
                nc.vector.tensor_copy(kt2[:, pt * P : (pt + 1) * P], tp[:])

            k_pair(0)
            k_pair(1)
            q_group(0)
            for qq in range(1, 4):
                k_pair(2 * qq)
                k_pair(2 * qq + 1)
            q_group(1)
            nc.vector.tensor_copy(
                vp3[:, :, 0:V], vstg[:].rearrange("p (t v) -> p t v", t=T)
            )
            q_group(2)
            q_group(3)

            vals_all = constp.tile([V, NO], f32)
            wo4 = weights_o.ap().rearrange("(g p two) o -> p g two o", g=NP, p=P)

            def values_block(span):
                # values = vals * rsqrt(sum_v vals^2); softmax denoms cancel
                o0b, cwb = span
                csl = slice(o0b, o0b + cwb)
                sq = smallp.tile([V, CH], bf16, tag="sq")
                nc.vector.tensor_mul(
                    sq[:, 0:cwb], vals_all[:, csl], vals_all[:, csl]
                )
                ss_ps = ps_m.tile([1, CH], f32, tag="misc")
                nc.tensor.matmul(
                    ss_ps[:, 0:cwb], ones_col[:, :], sq[:, 0:cwb],
                    start=True, stop=True,
                )
                lnt = smallp.tile([1, CH], f32, tag="lnt")
                nc.scalar.activation(lnt[:, 0:cwb], ss_ps[:, 0:cwb], AF.Ln)
                invn = smallp.tile([1, CH], bf16, tag="invn")
                nc.scalar.activation(
                    invn[:, 0:cwb], lnt[:, 0:cwb], AF.Exp, scale=-0.5
                )
                repn_ps = ps_m.tile([V, CH], f32, tag="misc")
                nc.tensor.matmul(
                    repn_ps[:, 0:cwb], ones_row[:, 0:V], invn[:, 0:cwb],
                    start=True, stop=True,
                )
                vouts = smallp.tile([V, CH], f32, tag="vouts")
                nc.vector.tensor_mul(
                    vouts[:, 0:cwb], vals_all[:, csl], repn_ps[:, 0:cwb]
                )
                nc.sync.dma_start(values_o.ap()[:, csl], vouts[:, 0:cwb])

            # chunk plan: narrow leading chunks so the weights DMA starts early
            CHUNKS = [(0, 512), (512, 512), (1024, 512), (1536, 512)]
            prev_recip = None
            for c, (o0, cw) in enumerate(CHUNKS):
                u = up.tile([P, T * cw], bf16, tag="u")
                u3 = u[:].rearrange("p (t n) -> p t n", t=T)
                vals_ps = ps_v.tile([V + 1, cw], f32, tag="vals")
                qsl = slice(o0, o0 + cw)
                for pt in range(NP):
                    # concurrent row-tiled matmuls must land in different PSUM
                    # banks: pin halves at CH-sized bank offsets even when
                    # cw < CH
                    s_ps = ps_s.tile([P, 2 * CH], f32, tag="s")
                    for j in range(2):
                        nc.tensor.matmul(
                            s_ps[:, j * CH : j * CH + cw],
                            kt2[j * K : (j + 1) * K, pt * P : (pt + 1) * P],
                            qt2[j * K : (j + 1) * K, qsl],
                            start=True,
                            stop=True,
                        )
                    exp_inst = nc.scalar.activation(
                        u3[:, 2 * pt : 2 * pt + 2, :],
                        s_ps[:].rearrange("p (j n) -> p j n", j=2)[:, :, 0:cw],
                        AF.Exp,
                        scale=0.125,
                    )
                    if pt == 0 and prev_recip is not None:
                        # keep the previous chunk's weights-DMA-gating recip
                        # ahead of this chunk's exps in the ACT stream
                        _add_dep_helper(
                            exp_inst.ins, prev_recip.ins,
                            sync=False, reason="recip before next-chunk exps",
                        )
                    for j in range(2):
                        t = 2 * pt + j
                        nc.tensor.matmul(
                            vals_ps[:],
                            vp3[:, t, :],
                            u3[:, t, :],
                            start=(t == 0),
                            stop=(t == T - 1),
                        )
                    if c >= 1 and pt % 2 == 1:
                        filler_mm()

                # weights = U * (1/colsum); recip via exp(-ln) stays in the exp
                # table set (ACT Reciprocal banned; DVE recip on 1 partition is
                # 3.4us). High priority: this chain gates the weights DMA, so
                # it must beat the next chunk's exps to the ACT queue.
                lncs = smallp.tile([1, CH], f32, tag="lncs")
                nc.scalar.activation(lncs[:, 0:cw], vals_ps[V : V + 1, :], AF.Ln)
                recip = smallp.tile([1, CH], bf16, tag="recip")
                prev_recip = nc.scalar.activation(
                    recip[:, 0:cw], lncs[:, 0:cw], AF.Exp, scale=-1.0
                )
                rep_ps = ps_m.tile([P, CH], f32, tag="misc")
                nc.tensor.matmul(
                    rep_ps[:, 0:cw], ones_row[:, :], recip[:, 0:cw],
                    start=True, stop=True,
                )
                rep = smallp.tile([P, CH], bf16, tag="rep")
                nc.scalar.copy(rep[:, 0:cw], rep_ps[:, 0:cw])

                w = wp.tile([P, T * CH], bf16, tag="w")
                w3 = w[:].rearrange("p (t n) -> p t n", t=T)[:, :, 0:cw]
                w4 = w[:].rearrange("p (g two n) -> p g two n", g=NP, two=2)[
                    :, :, :, 0:cw
                ]
                u3c = u3
                QT_, QG_ = T // 4, NP // 4
                for h in range(4):
                    tsl = slice(h * QT_, (h + 1) * QT_)
                    gsl = slice(h * QG_, (h + 1) * QG_)
                    rep_b = rep[:, 0:cw].unsqueeze(1).broadcast_to([P, QT_, cw])
                    nc.vector.tensor_mul(w3[:, tsl, :], u3c[:, tsl, :], rep_b)
                    for jj in range(2):
                        nc.gpsimd.dma_start(
                            wo4[:, gsl, jj, o0 : o0 + cw], w4[:, gsl, jj, :]
                        )

                # stash unnormalized vals; normalize one chunk behind so the
                # small ACT/PE/DVE ops slot into idle space off the DMA path
                nc.vector.tensor_copy(vals_all[:, qsl], vals_ps[0:V, :])
                filler_mm(3)
                if c >= 1:
                    values_block(CHUNKS[c - 1])
            values_block(CHUNKS[-1])

            # keep the verifier happy: PSUM locations need a reader
            nc.vector.tensor_copy(wu_src[0:1, 0:1], wu_ps[0:1, 0:1])

    nc.compile()
    return nc


def get_nc():
    if "nc" not in _CACHE:
        _CACHE["nc"] = _build()
    return _CACHE["nc"]


def kernel(key, query, value):
    from concourse.bass_utils import run_bass_kernel_spmd

    key = np.ascontiguousarray(np.asarray(key, dtype=np.float32))
    query = np.ascontiguousarray(np.asarray(query, dtype=np.float32))
    value = np.ascontiguousarray(np.asarray(value, dtype=np.float32))

    nc = get_nc()
    in_maps = [
        {
            "key": np.ascontiguousarray(key[i, :, 0, :]),
            "query": np.ascontiguousarray(query[i, 0]),
            "value": np.ascontiguousarray(value[i]),
        }
        for i in range(B)
    ]
    res = run_bass_kernel_spmd(nc, in_maps, core_ids=list(range(B)))
    values = np.stack([res.results[i]["values_out"] for i in range(B)])
    weights = np.stack([res.results[i]["weights_out"] for i in range(B)])
    return values, weights
